# revision 2
# baseline (speedup 1.0000x reference)
"""Causal self-attention (B=2, T=2048, dim=2048, H=16, D=128) on 8 trn2 NeuronCores.

Sharding: data-parallel over batch (2 groups of 4 cores), tensor-parallel over
heads within a group (4 heads/core).  Each core computes its heads' QKV
projection (x @ Wqkv_part^T), RoPE, causal attention, and a partial output
projection against its W_proj column block; the host sums the 4 partials per
batch element.

v2: fused per-window pipeline.  The T=2048 sequence is processed in 4 windows
of 512 queries; for each window the kernel emits QKV projection, RoPE,
causal attention (against all keys <= window end), and the output projection
for that window's token tiles.  This lets the Tile scheduler overlap DVE/
ScalarE/DMA work of one window with TensorE work of the neighbouring windows
and keeps the PE array continuously busy (HAM stays warm).

Other deltas vs v1:
  - softmax rowsum matmul uses a ones(128x128) stationary so the rowsums
    come out pre-broadcast across partitions (one reciprocal + one multiply,
    no separate K=1 broadcast matmul / copy).
  - cos/sin tables in bf16 -> RoPE tensor ops hit the DVE 2x packed mode.
  - weight/x DMAs split per 128-row chunk so the first matmuls start early.
"""

import os

import numpy as np
import ml_dtypes

import concourse.bass as bass
import concourse.bacc as bacc
import concourse.tile as tile
import concourse.mybir as mybir
from concourse import bass_utils

BF16 = mybir.dt.bfloat16
F32 = mybir.dt.float32

B, T, DIM = 2, 2048, 2048
H, D = 16, 128
HL = 4                   # heads per core
NCORES = 8
E = 3 * HL * D           # 1536 = per-core qkv output rows
NCHUNK = DIM // 128      # 16 contraction chunks
NW = T // 512            # 4 query windows
NTT = T // 128           # 16 token tiles
SCALE = 1.0 / float(np.sqrt(D))

_CACHE = {}
LAST_RESULTS = None

# rowsum strategy: if True, accumulate exp tiles elementwise on DVE (bf16)
# and do a single ones-matmul partition-reduce per (head, window); if False,
# accumulate rowsums on the PE (3rd matmul stream per key tile).
ROWSUM_DVE = os.environ.get("ROWSUM_DVE", "1") == "1"

# Q/K projections in fp8(e4m3) with DoubleRow (K packed 2/partition -> 8
# chunks of 256).  V projection / attention / out-proj stay bf16: pre-softmax
# score noise is smoothed by softmax+averaging (~0.2% output impact) while
# post-softmax fp8 would hit the output directly (~3%).
QK_FP8 = os.environ.get("QK_FP8", "0") == "1"
FP8 = mybir.dt.float8e4
WQK_SCALE = 256.0

# interleave attention head pairs (two independent QK->exp->PV chains in
# flight hides cross-engine semaphore latency)
HEAD_PAIR = os.environ.get("HEAD_PAIR", "0") == "1"

# unroll factor for the loop_iters timing build (measures For_i boundary cost)
UNROLL = int(os.environ.get("UNROLL", "1"))

# engine rebalancing knobs
YCOPY_ALT = os.environ.get("YCOPY_ALT", "0") == "1"   # alternate y copies DVE/ScalarE
PACC_GPS = os.environ.get("PACC_GPS", "0") == "1"     # pacc adds for heads 2,3 on gpsimd
ROPE_GPS = os.environ.get("ROPE_GPS", "0") == "1"     # k-RoPE muls on gpsimd

# phase stripping for perf attribution: "q"=qkv only, "qr"=+rope,
# "qra"=+attention, "all"=+proj (default)
PHASES = os.environ.get("PHASES", "all")

# batch DMAs: one xb DMA per steady-state window, 4-head rot swaps, per-row y
BATCH_DMA = os.environ.get("BATCH_DMA", "1") == "1"

# fold the causal mask into the scores (add -1e30 upper-tri via one extra
# 128-col matmul into the diag tile) instead of multiplying exp() by a 0/1
# triangle on the DVE
MASK_MM = os.environ.get("MASK_MM", "0") == "1"

# share one 2-slot PSUM tag between the rowsum tiles and the proj tiles so
# the projection is double-buffered in every window
RY_SHARED = os.environ.get("RY_SHARED", "0") == "1"

PT_BUFS = int(os.environ.get("PT_BUFS", "3"))
ST_BUFS = int(os.environ.get("ST_BUFS", "2"))
# batch RoPE tensor ops over all 4 heads against head-replicated cos/sin
ROPE_BATCH = os.environ.get("ROPE_BATCH", "0") == "1"
# rowsum-broadcast tiles allocated from the st tag (frees a PSUM bank)
RS_ST = os.environ.get("RS_ST", "0") == "1"


def _build_module(loop_iters=1):
    nc = bacc.Bacc("TRN2", target_bir_lowering=False, debug=False)
    xT = nc.dram_tensor("xT", (DIM, T), BF16, kind="ExternalInput")
    if QK_FP8:
        x8d = nc.dram_tensor("x8", (128, 8, 2, T), FP8, kind="ExternalInput")
        w8d = nc.dram_tensor("w8", (128, 8, 2, 1024), FP8, kind="ExternalInput")
        wvd = nc.dram_tensor("wv", (128, NCHUNK, 512), BF16, kind="ExternalInput")
    else:
        wqkvT = nc.dram_tensor("wqkvT", (DIM, E), BF16, kind="ExternalInput")
    wpT = nc.dram_tensor("wpT", (HL * D, DIM), BF16, kind="ExternalInput")
    cosT = nc.dram_tensor("cosT", (D, T), BF16, kind="ExternalInput")
    sinTs = nc.dram_tensor("sinTs", (D, T), BF16, kind="ExternalInput")
    tri = nc.dram_tensor("tri", (128, 128), BF16, kind="ExternalInput")
    ones_bc = nc.dram_tensor("ones_bc", (128, 128), BF16, kind="ExternalInput")
    if MASK_MM:
        idm = nc.dram_tensor("idm", (128, 128), BF16, kind="ExternalInput")
        mnegd = nc.dram_tensor("mnegd", (128, 128), BF16, kind="ExternalInput")
    y = nc.dram_tensor("y", (T, DIM), BF16, kind="ExternalOutput")

    Exp = mybir.ActivationFunctionType.Exp

    with tile.TileContext(nc) as tc:
        with (
            tc.tile_pool(name="const", bufs=1) as cpool,
            tc.tile_pool(name="xp", bufs=2) as xpool,
            tc.tile_pool(name="qp", bufs=2) as qpool,
            tc.tile_pool(name="op", bufs=2) as opool,
            tc.tile_pool(name="rotp", bufs=2) as rotpool,
            tc.tile_pool(name="ptp", bufs=PT_BUFS) as ptpool,
            tc.tile_pool(name="rcpp", bufs=2) as rcppool,
            tc.tile_pool(name="paccp", bufs=2) as paccpool,
            tc.tile_pool(name="yp", bufs=2) as ypool,
            tc.tile_pool(name="qkps", bufs=2, space="PSUM") as qkps,
            tc.tile_pool(name="stps", bufs=ST_BUFS, space="PSUM") as stps,
            tc.tile_pool(name="otps", bufs=2, space="PSUM") as otps,
            tc.tile_pool(name="rsps", bufs=2 if RY_SHARED else 1, space="PSUM") as rsps,
            tc.tile_pool(name="yps", bufs=1, space="PSUM") as yps,
        ):
            # ---- constants / persistent tensors ----
            # (weight DMA is interleaved with window-0 x DMA inside _phases
            #  so the first matmuls start as soon as chunk 0 lands; cos/sin/
            #  tri/ones/wp are emitted just before their first use.)
            if QK_FP8:
                w8_sb = cpool.tile([128, 8, 2, 1024], FP8, tag="w8")
                wv_sb = cpool.tile([128, NCHUNK, 512], BF16, tag="wv")
            else:
                w_sb = cpool.tile([128, NCHUNK, E], BF16, tag="w")
                wqkvT_v = wqkvT.rearrange("(c p) e -> p c e", p=128)
            xT_v = xT.rearrange("(c p) t -> p c t", p=128)
            if ROPE_BATCH:
                cos_sb = cpool.tile([128, HL, T], BF16, tag="cos")
                sin_sb = cpool.tile([128, HL, T], BF16, tag="sin")
            else:
                cos_sb = cpool.tile([128, T], BF16, tag="cos")
                sin_sb = cpool.tile([128, T], BF16, tag="sin")
            tri_sb = cpool.tile([128, 128], BF16, tag="tri")
            ones_sb = cpool.tile([128, 128], BF16, tag="ones")
            if MASK_MM:
                id_sb = cpool.tile([128, 128], BF16, tag="idm")
                mneg_sb = cpool.tile([128, 128], BF16, tag="mneg")
            wp_sb = cpool.tile([128, HL, DIM], BF16, tag="wp")

            # full-length K / V caches (all windows)
            k_sb = cpool.tile([128, HL, T], BF16, tag="k")
            v_sb = cpool.tile([128, NTT * HL * D], BF16, tag="v")

            def _load_consts():
                if ROPE_BATCH:
                    for j in range(HL):
                        nc.sync.dma_start(cos_sb[:, j, :], cosT[:, :])
                        nc.sync.dma_start(sin_sb[:, j, :], sinTs[:, :])
                else:
                    nc.sync.dma_start(cos_sb[:], cosT[:, :])
                    nc.sync.dma_start(sin_sb[:], sinTs[:, :])
                nc.sync.dma_start(tri_sb[:], tri[:, :])
                nc.sync.dma_start(ones_sb[:], ones_bc[:, :])
                if MASK_MM:
                    nc.sync.dma_start(id_sb[:], idm[:, :])
                    nc.sync.dma_start(mneg_sb[:], mnegd[:, :])
                nc.sync.dma_start(wp_sb[:], wpT.rearrange("(h p) n -> p h n", p=128))

            DR = mybir.MatmulPerfMode.DoubleRow

            def qk_chain(ps, base, x8b, xb):
                if QK_FP8:
                    for c in range(8):
                        nc.tensor.matmul(
                            ps[:],
                            w8_sb[:, c, :, base : base + 128],
                            x8b[:, c, :, :],
                            start=(c == 0),
                            stop=(c == 7),
                            perf_mode=DR,
                        )
                else:
                    for c in range(NCHUNK):
                        nc.tensor.matmul(
                            ps[:],
                            w_sb[:, c, base : base + 128],
                            xb[:, c, :],
                            start=(c == 0),
                            stop=(c == NCHUNK - 1),
                        )

            def qk_evict(dst, ps):
                if QK_FP8:
                    nc.scalar.mul(dst, ps[:], 1.0 / WQK_SCALE)
                else:
                    nc.scalar.copy(dst, ps[:])

            def _phases(first=False):
                for w in range(NW):
                    t0 = w * 512
                    # ---- QKV projection for this window ----
                    xb = xpool.tile([128, NCHUNK, 512], BF16, tag="x")
                    x8b = None
                    if QK_FP8:
                        x8b = xpool.tile([128, 8, 2, 512], FP8, tag="x8")
                        for c in range(8):
                            nc.sync.dma_start(
                                x8b[:, c, :, :], x8d[:, c, :, t0 : t0 + 512]
                            )
                            if first and w == 0:
                                nc.sync.dma_start(w8_sb[:, c, :, :], w8d[:, c, :, :])
                        for c in range(NCHUNK):
                            nc.sync.dma_start(xb[:, c, :], xT_v[:, c, t0 : t0 + 512])
                            if first and w == 0:
                                nc.sync.dma_start(wv_sb[:, c, :], wvd[:, c, :])
                    elif BATCH_DMA and not (first and w == 0):
                        nc.sync.dma_start(xb[:], xT_v[:, :, t0 : t0 + 512])
                    else:
                        for c in range(NCHUNK):
                            nc.sync.dma_start(xb[:, c, :], xT_v[:, c, t0 : t0 + 512])
                            if first and w == 0:
                                nc.sync.dma_start(w_sb[:, c, :], wqkvT_v[:, c, :])
                    if first and w == 0:
                        _load_consts()
                    q_win = qpool.tile([128, HL, 512], BF16, tag="q")
                    # q and k: out (e-tile 128, t 512), e on partitions (transposed)
                    if first and w == 0:
                        # single-shot first window: the x/w chunk DMAs pace the
                        # PE.  Run all 8 q/k chains c-major across 8 borrowed
                        # PSUM banks so in-order PE work tracks chunk arrivals.
                        chains = [
                            pool.tile([128, 512], F32, tag=tg, name=f"ch{i}")
                            for i, (pool, tg) in enumerate(
                                [
                                    (qkps, "qk"),
                                    (qkps, "qk"),
                                    (stps, "st"),
                                    (stps, "st"),
                                    (otps, "oT"),
                                    (otps, "oT"),
                                    (stps, "st") if RS_ST
                                    else (rsps, "ry" if RY_SHARED else "rs"),
                                    (rsps, "ry") if RY_SHARED else (yps, "y"),
                                ]
                            )
                        ]
                        nchn = 8 if QK_FP8 else NCHUNK
                        for c in range(nchn):
                            for i in range(8):
                                if QK_FP8:
                                    nc.tensor.matmul(
                                        chains[i][:],
                                        w8_sb[:, c, :, i * 128 : (i + 1) * 128],
                                        x8b[:, c, :, :],
                                        start=(c == 0),
                                        stop=(c == nchn - 1),
                                        perf_mode=DR,
                                    )
                                else:
                                    nc.tensor.matmul(
                                        chains[i][:],
                                        w_sb[:, c, i * 128 : (i + 1) * 128],
                                        xb[:, c, :],
                                        start=(c == 0),
                                        stop=(c == nchn - 1),
                                    )
                        for j in range(HL):
                            qk_evict(q_win[:, j, :], chains[j])
                        for j in range(HL):
                            qk_evict(
                                k_sb[:, j, t0 : t0 + 512],
                                chains[HL + j],
                            )
                    else:
                        for grp in range(2):
                            for j in range(HL):
                                ps = qkps.tile([128, 512], F32, tag="qk")
                                base = grp * 512 + j * 128
                                qk_chain(ps, base, x8b, xb)
                                if grp == 0:
                                    qk_evict(q_win[:, j, :], ps)
                                else:
                                    qk_evict(
                                        k_sb[:, j, t0 : t0 + 512], ps
                                    )
                    # v: out (t-tile 128, e 512), natural layout
                    for ttl in range(4):
                        ttg = w * 4 + ttl
                        ps = qkps.tile([128, 512], F32, tag="qk")
                        for c in range(NCHUNK):
                            nc.tensor.matmul(
                                ps[:],
                                xb[:, c, ttl * 128 : (ttl + 1) * 128],
                                wv_sb[:, c, :] if QK_FP8 else w_sb[:, c, 1024:1536],
                                start=(c == 0),
                                stop=(c == NCHUNK - 1),
                            )
                        nc.scalar.copy(v_sb[:, ttg * 512 : (ttg + 1) * 512], ps[:])

                    if PHASES == "q":
                        continue
                    # ---- RoPE on this window's q, k (in place) ----
                    for src_is_k in range(2):
                        eng = nc.gpsimd if (ROPE_GPS and src_is_k) else nc.vector
                        if ROPE_BATCH:
                            src4 = (
                                k_sb[:, :, t0 : t0 + 512]
                                if src_is_k
                                else q_win[:, :, :]
                            )
                            rot4 = rotpool.tile([128, HL, 512], BF16, tag="rot4")
                            nc.sync.dma_start(rot4[0:64, :, :], src4[64:128, :, :])
                            nc.sync.dma_start(rot4[64:128, :, :], src4[0:64, :, :])
                            eng.tensor_mul(
                                rot4[:], rot4[:], sin_sb[:, :, t0 : t0 + 512]
                            )
                            eng.tensor_mul(
                                src4, src4, cos_sb[:, :, t0 : t0 + 512]
                            )
                            eng.tensor_add(src4, src4, rot4[:])
                            continue
                        if BATCH_DMA:
                            src4 = (
                                k_sb[:, :, t0 : t0 + 512]
                                if src_is_k
                                else q_win[:, :, :]
                            )
                            rot4 = rotpool.tile([128, HL, 512], BF16, tag="rot4")
                            nc.sync.dma_start(rot4[0:64, :, :], src4[64:128, :, :])
                            nc.sync.dma_start(rot4[64:128, :, :], src4[0:64, :, :])
                        for h in range(HL):
                            if src_is_k:
                                s_ap = k_sb[:, h, t0 : t0 + 512]
                            else:
                                s_ap = q_win[:, h, :]
                            if BATCH_DMA:
                                rot = rot4[:, h, :]
                            else:
                                rott = rotpool.tile([128, 512], BF16, tag="rot")
                                nc.sync.dma_start(rott[0:64, :], s_ap[64:128, :])
                                nc.sync.dma_start(rott[64:128, :], s_ap[0:64, :])
                                rot = rott[:]
                            eng.tensor_mul(
                                rot, rot, sin_sb[:, t0 : t0 + 512]
                            )
                            eng.tensor_mul(
                                s_ap, s_ap, cos_sb[:, t0 : t0 + 512]
                            )
                            eng.tensor_add(s_ap, s_ap, rot)

                    if PHASES == "qr":
                        continue
                    # ---- causal attention for this query window ----
                    o_win = opool.tile([128, HL, 512], BF16, tag="o")
                    nkt = 4 * w + 4
                    if HEAD_PAIR and ROWSUM_DVE:
                        head_groups = [(0, 1), (2, 3)]
                    else:
                        head_groups = [(h,) for h in range(HL)]
                    for hs in head_groups:
                        oT_ps = {}
                        pacc = {}
                        rs_ps = {}
                        for h in hs:
                            oT_ps[h] = otps.tile([128, 512], F32, tag="oT", name=f"oT{h}")
                            if ROWSUM_DVE:
                                pacc[h] = paccpool.tile(
                                    [128, 512], BF16, tag="pacc", name=f"pacc{h}"
                                )
                            else:
                                rs_ps[h] = rsps.tile(
                                    [128, 512], F32, tag="rs", name=f"rs{h}"
                                )
                        for kt in range(nkt):
                            if kt < 4 * w:
                                off, n, diag = 0, 512, False
                            else:
                                off = 128 * kt - t0
                                n = 512 - off
                                diag = True
                            for h in hs:
                                hq = h * T
                                st = stps.tile([128, 512], F32, tag="st")
                                if diag and MASK_MM:
                                    nc.tensor.matmul(
                                        st[:, 0:128],
                                        id_sb[:],
                                        mneg_sb[:],
                                        start=True,
                                        stop=False,
                                        skip_group_check=True,
                                    )
                                nc.tensor.matmul(
                                    st[:, :n],
                                    k_sb[:, h, kt * 128 : (kt + 1) * 128],
                                    q_win[:, h, off:512],
                                    start=not (diag and MASK_MM),
                                    stop=True,
                                    skip_group_check=(diag and MASK_MM),
                                )
                                pt = ptpool.tile([128, 512], BF16, tag="pt")
                                nc.scalar.activation(
                                    pt[:, :n], st[:, :n], Exp, bias=0.0, scale=SCALE
                                )
                                if diag and not MASK_MM:
                                    nc.vector.tensor_mul(
                                        pt[:, 0:128], pt[:, 0:128], tri_sb[:]
                                    )
                                nc.tensor.matmul(
                                    oT_ps[h][:, off:512],
                                    v_sb[
                                        :,
                                        kt * 512 + h * 128 : kt * 512 + (h + 1) * 128,
                                    ],
                                    pt[:, :n],
                                    start=(kt == 0),
                                    stop=(kt == nkt - 1),
                                )
                                if ROWSUM_DVE:
                                    peng = (
                                        nc.gpsimd
                                        if (PACC_GPS and h >= 2)
                                        else nc.vector
                                    )
                                    if kt == 0:
                                        peng.tensor_copy(pacc[h][:], pt[:])
                                    else:
                                        peng.tensor_add(
                                            pacc[h][:, off:512],
                                            pacc[h][:, off:512],
                                            pt[:, :n],
                                        )
                                else:
                                    nc.tensor.matmul(
                                        rs_ps[h][:, off:512],
                                        ones_sb[:],
                                        pt[:, :n],
                                        start=(kt == 0),
                                        stop=(kt == nkt - 1),
                                    )
                        for h in hs:
                            if ROWSUM_DVE:
                                if RS_ST:
                                    rs = stps.tile(
                                        [128, 512], F32, tag="st", name=f"rsb{h}"
                                    )
                                else:
                                    rs = rsps.tile(
                                        [128, 512],
                                        F32,
                                        tag="ry" if RY_SHARED else "rs",
                                        name=f"rsb{h}",
                                    )
                                nc.tensor.matmul(
                                    rs[:], ones_sb[:], pacc[h][:], start=True, stop=True
                                )
                            else:
                                rs = rs_ps[h]
                            rcp = rcppool.tile([128, 512], F32, tag="rcp")
                            nc.vector.reciprocal(rcp[:], rs[:])
                            nc.vector.tensor_mul(o_win[:, h, :], oT_ps[h][:], rcp[:])

                    if PHASES == "qra":
                        continue
                    # ---- output projection for this window's token tiles ----
                    for ttl in range(4):
                        tt = w * 4 + ttl
                        if BATCH_DMA:
                            ysb4 = ypool.tile([128, 4, 512], BF16, tag="ysb4")
                        for nw in range(DIM // 512):
                            # last window: alternate with the (now idle) qkv
                            # PSUM tag so the PSUM->SBUF copy double-buffers
                            if RY_SHARED:
                                yp = rsps.tile([128, 512], F32, tag="ry", name="ypt")
                            elif w == NW - 1 and (ttl * 4 + nw) % 2 == 1:
                                yp = qkps.tile([128, 512], F32, tag="qk")
                            else:
                                yp = yps.tile([128, 512], F32, tag="y")
                            for hh in range(HL):
                                nc.tensor.matmul(
                                    yp[:],
                                    o_win[:, hh, ttl * 128 : (ttl + 1) * 128],
                                    wp_sb[:, hh, nw * 512 : (nw + 1) * 512],
                                    start=(hh == 0),
                                    stop=(hh == HL - 1),
                                )
                            if BATCH_DMA:
                                ysb = ysb4[:, nw, :]
                            else:
                                ysbt = ypool.tile([128, 512], BF16, tag="ysb")
                                ysb = ysbt[:]
                            if YCOPY_ALT and (ttl * 4 + nw) % 2 == 1:
                                nc.scalar.copy(ysb, yp[:])
                            else:
                                nc.vector.tensor_copy(ysb, yp[:])
                            if not BATCH_DMA:
                                nc.sync.dma_start(
                                    y[
                                        tt * 128 : (tt + 1) * 128,
                                        nw * 512 : (nw + 1) * 512,
                                    ],
                                    ysb,
                                )
                        if BATCH_DMA:
                            nc.sync.dma_start(y[tt * 128 : (tt + 1) * 128, :], ysb4[:])

            if loop_iters > 1:
                if QK_FP8:
                    for c in range(8):
                        nc.sync.dma_start(w8_sb[:, c, :, :], w8d[:, c, :, :])
                    for c in range(NCHUNK):
                        nc.sync.dma_start(wv_sb[:, c, :], wvd[:, c, :])
                else:
                    for c in range(NCHUNK):
                        nc.sync.dma_start(w_sb[:, c, :], wqkvT_v[:, c, :])
                _load_consts()
                assert loop_iters % UNROLL == 0
                with tc.For_i(0, loop_iters // UNROLL, 1):
                    for _ in range(UNROLL):
                        _phases(first=False)
            else:
                _phases(first=True)
    nc.compile()
    return nc


def _rope_tables():
    inv_freq = (
        1.0 / (10000.0 ** (np.arange(0, D, 2, dtype=np.float32) / np.float32(D)))
    ).astype(np.float32)
    tpos = np.arange(T, dtype=np.float32)
    freqs = tpos[:, None] * inv_freq[None, :]
    emb = np.concatenate([freqs, freqs], axis=1)  # (T, D)
    cos = np.cos(emb).astype(np.float32)
    sin = np.sin(emb).astype(np.float32)
    cosT = np.ascontiguousarray(cos.T)  # (D, T)
    sinTs = np.ascontiguousarray(sin.T)
    sinTs[0:64] *= -1.0  # fold rotate_half sign
    return cosT, sinTs


def make_in_maps(x, W_qkv, W_proj):
    cosT, sinTs = _rope_tables()
    cosT = cosT.astype(ml_dtypes.bfloat16)
    sinTs = sinTs.astype(ml_dtypes.bfloat16)
    tri = (np.arange(128)[None, :] >= np.arange(128)[:, None]).astype(
        ml_dtypes.bfloat16
    )
    tri = np.ascontiguousarray(tri)
    ones_bc = np.ones((128, 128), dtype=ml_dtypes.bfloat16)
    in_maps = []
    for c in range(NCORES):
        b, g = divmod(c, 4)
        Wq = W_qkv[512 * g : 512 * (g + 1)]
        Wk = W_qkv[2048 + 512 * g : 2048 + 512 * (g + 1)]
        Wv = W_qkv[4096 + 512 * g : 4096 + 512 * (g + 1)]
        xT_f = np.ascontiguousarray(x[b].T)  # (DIM, T) f32
        m = {
            "xT": xT_f.astype(ml_dtypes.bfloat16),
            "wpT": np.ascontiguousarray(
                W_proj[:, 512 * g : 512 * (g + 1)].T
            ).astype(ml_dtypes.bfloat16),
            "cosT": cosT,
            "sinTs": sinTs,
            "tri": tri,
            "ones_bc": ones_bc,
        }
        if MASK_MM:
            m["idm"] = np.eye(128, dtype=ml_dtypes.bfloat16)
            mneg = np.where(
                np.arange(128)[None, :] >= np.arange(128)[:, None], 0.0, -1e30
            ).astype(ml_dtypes.bfloat16)
            m["mnegd"] = np.ascontiguousarray(mneg)
        if QK_FP8:
            # x8: (DIM, T) -> [c, p, i, t] with dim = 256c + 2p + i -> (128, 8, 2, T)
            m["x8"] = np.ascontiguousarray(
                xT_f.reshape(8, 128, 2, T).transpose(1, 0, 2, 3)
            ).astype(ml_dtypes.float8_e4m3)
            wqkT = np.concatenate([Wq, Wk], axis=0).T * WQK_SCALE  # (DIM, 1024)
            m["w8"] = np.ascontiguousarray(
                wqkT.reshape(8, 128, 2, 1024).transpose(1, 0, 2, 3)
            ).astype(ml_dtypes.float8_e4m3)
            # wv: (DIM, 512) -> [p, c, e]
            m["wv"] = np.ascontiguousarray(
                Wv.T.reshape(NCHUNK, 128, 512).transpose(1, 0, 2)
            ).astype(ml_dtypes.bfloat16)
        else:
            Wc = np.concatenate([Wq, Wk, Wv], axis=0)  # (1536, 2048)
            m["wqkvT"] = np.ascontiguousarray(Wc.T).astype(ml_dtypes.bfloat16)
        in_maps.append(m)
    return in_maps


def postprocess_core_output(result):
    return np.asarray(result["y"]).astype(np.float32)


def kernel(x, W_qkv, W_proj):
    global LAST_RESULTS
    x = np.asarray(x, dtype=np.float32)
    W_qkv = np.asarray(W_qkv, dtype=np.float32)
    W_proj = np.asarray(W_proj, dtype=np.float32)
    assert x.shape == (B, T, DIM) and W_qkv.shape == (3 * H * D, DIM)

    if "nc" not in _CACHE:
        _CACHE["nc"] = _build_module()
    nc = _CACHE["nc"]

    in_maps = make_in_maps(x, W_qkv, W_proj)
    trace = os.environ.get("KERNEL_TRACE", "0") == "1"
    res = bass_utils.run_bass_kernel_spmd(
        nc, in_maps, core_ids=list(range(NCORES)), trace=trace
    )
    LAST_RESULTS = res
    y = np.zeros((B, T, DIM), dtype=np.float32)
    for c in range(NCORES):
        y[c // 4] += postprocess_core_output(res.results[c])
    return y
